# revision 46
# baseline (speedup 1.0000x reference)
"""LQLinear (2-bit learned VQ linear) Trainium2 kernel.

Math (Q_T=1): the least-squares basis refit only feeds the *discarded*
buffer update, so the forward output is

    out = x @ wq.T + bias

where wq bucketizes weight into the 4 sorted levels {+-b_small, +-3*b_small}
(b_big = 2*b_small for the reference basis), thresholds {-b_big, 0, +b_big}.

Device strategy (8 cores = 4 out-feature shards x 2 token shards):
  - per core: 1024 out rows x 4096 tokens, full K=4096.
  - wq = b_small * wqn, wqn = sign(w)*(R + sign(|w|-b_big)) in {+-1,+-3}
    (R = b_big/b_small = 2): EXACT in fp8e4m3. Quantize runs on ACT+DVE
    from f32 weights (f32 compare needed: bf16 weights misclassify
    ~3e-4 of weights at the +-b_big thresholds -> ~1.5% output error).
    The |w| vs b_big compare is done as w^2 vs b_big^2 so the abs step
    is a plain DVE multiply (walrus rejects abs_max in tensor ops);
    the final (ss2+R)*s_big is one fused DVE scalar_tensor_tensor.
    Quantize streams per-osb (128 output cols at a time, osb0 in
    quarter-K chunks) so the GEMM starts consuming wq after ~10us.
  - GEMM in fp8 DoubleRow perf mode (2 k-tiles per instruction, 0.5
    cyc/row = 157 TF/s): x split host-side into fp8 hi + fp8 lo
    (x = hi + lo to ~7 mantissa bits); both streams accumulate into one
    PSUM group. Measured rel err ~1.8e-3 incl. bf16 out (gate 2e-2).
    LQ_MODE=bf16 is a fallback single-stream bf16 path.
  - Token blocks run in pairs, osb-major inside the pair, so the PE has
    two token blocks of work per wq column block while phase A streams.
  - All DMAs are large contiguous per-partition chunks (8KB
    descriptors, 128 descriptors/transfer) instead of the v1 2KB
    patterns, spread over three queues: x-hi on the SP HWDGE, x-lo on
    the ACT HWDGE, weights + outputs on the Pool SWDGE (gpsimd).
  - DVE evicts PSUM with fused out = b_small*psum + bias[o], stored
    bf16 (halves output traffic; host upcasts).
  - Host prep is layout/cast-only sharding work: transpose/cast/slice.

TimelineSim cost-model estimate: ~300us vs 516us for the previous
f32r baseline; the bigger effect is real-HW DMA structure (the old
kernel issued ~78k 2KB descriptors on one HWDGE queue, this one ~4k
8KB descriptors across three queues).
"""

import os
import sys

for _p in ("/opt/trn_rl_repo", "/root/.axon_site/_ro/trn_rl_repo"):
    if os.path.isdir(_p) and _p not in sys.path:
        sys.path.insert(0, _p)

import numpy as np
import ml_dtypes

N_CORES = 8
TOKENS = 8192
IN_F = 4096
OUT_F = 4096

N_OSH = 4                            # out-feature shards
N_TSH = 2                            # token shards
O_SHARD = OUT_F // N_OSH             # 1024 out rows per core
T_SHARD = TOKENS // N_TSH            # 4096 tokens per core
KT = IN_F // 128                     # 32 k-tiles
KP = KT // 2                         # 16 k-tile pairs (DoubleRow)
TB = 512                             # token block (psum free dim)
N_TB = T_SHARD // TB                 # 8 token blocks per core
O_SUB = O_SHARD // 128               # 8 output subtiles per core

LAST_RUN_INFO = {}


def _build_nc(b_small: float, b_big: float, mode: str):
    import concourse.bass as bass
    import concourse.mybir as mybir
    import concourse.tile as tile
    from concourse import bacc

    dt = mybir.dt
    Alu = mybir.AluOpType
    R = b_big / b_small

    fp8 = mode == "fp8dr"
    xdt = dt.float8e4 if fp8 else dt.bfloat16

    nc = bacc.Bacc("TRN2", target_bir_lowering=False, debug=False,
                   dynamic_dma_scratch_size=8192)

    KH = KT // 2                     # k-tiles per half-K chunk
    wp = nc.dram_tensor("wp", [O_SUB, 2, 128, KH * 128], dt.float32,
                        kind="ExternalInput")
    xh = nc.dram_tensor("xh", [N_TB, 128, KT * TB], xdt,
                        kind="ExternalInput")
    if fp8:
        xl = nc.dram_tensor("xl", [N_TB, 128, KT * TB], xdt,
                            kind="ExternalInput")
    bs = nc.dram_tensor("bs", [128, O_SUB], dt.float32, kind="ExternalInput")
    oT = nc.dram_tensor("oT", [N_TB, 128, O_SUB * TB], dt.bfloat16,
                        kind="ExternalOutput")

    wp_r = wp.ap()
    xh_r = xh.ap()
    if fp8:
        xl_r = xl.ap()
    oT_r = oT.ap()

    DR = mybir.MatmulPerfMode.DoubleRow

    with tile.TileContext(nc) as tc:
        with (
            tc.tile_pool(name="const", bufs=1) as const,
            tc.tile_pool(name="wq", bufs=1) as wqp,
            tc.tile_pool(name="wload", bufs=2) as wload,
            tc.tile_pool(name="quant", bufs=2) as qp,
            tc.tile_pool(name="xhp", bufs=3) as xhp,
            tc.tile_pool(name="xlp", bufs=3) as xlp,
            tc.tile_pool(name="outp", bufs=2) as outp,
            tc.tile_pool(name="psum", bufs=8, space="PSUM") as psp,
        ):
            bias_sb = const.tile([128, O_SUB], dt.float32)
            nc.sync.dma_start(bias_sb[:], bs.ap())
            # threshold as -b_big^2: |w| > b_big is evaluated as w^2 > b_big^2
            # so the |w| step can run on DVE as a plain multiply
            nbb = const.tile([128, 1], dt.float32, tag="nbb")
            nc.vector.memset(nbb[:], -float(b_big) * float(b_big))

            # ---- Phase A: quantize weight shard -> wqn {+-1,+-R} fp8/bf16,
            # [k-partition, kt, o] layout for (DoubleRow) lhsT slices.
            # Ordered per-osb so each 128-wide output-column block of wq
            # completes early; the GEMM consumes blocks while later ones
            # are still quantizing.
            wq_sb = wqp.tile([128, KT, O_SHARD], xdt)

            def quant_chunk(osb, k0, nk, dma_engine):
                # quantize w[k-tiles k0:k0+nk, osb block] -> wq
                w_t = wload.tile([128, nk, 128], dt.float32, name="w_t",
                                 tag="wl")
                dma_engine.dma_start(
                    w_t[:], wp_r[osb, k0 // KH, :,
                                 (k0 % KH) * 128:(k0 % KH + nk) * 128])
                sb = qp.tile([128, nk, 128], dt.float32, name="sb", tag="sb")
                av = qp.tile([128, nk, 128], dt.float32, name="av", tag="av")
                # ACT: s_big = sign(w); DVE or Pool: w^2;
                # ACT: ss2 = sign(w^2 - b_big^2)  (== sign(|w| - b_big));
                # DVE fused: wqn = (ss2 + R) * s_big in {+-(R-1), +-(R+1)}
                nc.scalar.sign(sb[:], w_t[:])
                nc.vector.tensor_tensor(av[:], w_t[:], w_t[:], Alu.mult)
                nc.scalar.sign(av[:], av[:], bias=nbb[:])
                nc.vector.scalar_tensor_tensor(
                    wq_sb[:, k0:k0 + nk, osb * 128:(osb + 1) * 128],
                    av[:], R, sb[:], Alu.add, Alu.mult)

            for osb in range(O_SUB):
                if osb == 0:
                    # quarter-K chunks: lower latency to the first wq block
                    for q in range(4):
                        quant_chunk(0, q * (KH // 2), KH // 2, nc.gpsimd)
                else:
                    quant_chunk(osb, 0, KH, nc.gpsimd)
                    quant_chunk(osb, KH, KH, nc.gpsimd)

            # ---- Phase B: GEMM psum[o128, t512] += wqn.T @ x, DoubleRow.
            # Token blocks run in pairs, osb-major inside the pair: the PE
            # has two token blocks of work per wq column block, so it keeps
            # up while quantization streams in behind it.
            def x_tiles(tb):
                t = xhp.tile([128, KT, TB], xdt, tag="xh")
                nc.sync.dma_start(t[:], xh_r[tb])
                l = None
                if fp8:
                    l = xlp.tile([128, KT, TB], xdt, tag="xl")
                    nc.scalar.dma_start(l[:], xl_r[tb])
                return (t, l)

            def mm_group(ps, xt, osb):
                osl = slice(osb * 128, (osb + 1) * 128)
                xh_t, xl_t = xt
                if fp8:
                    for j in range(KP):  # ktpair j
                        lhsT = wq_sb[:, 2 * j:2 * j + 2, osl]
                        nc.tensor.matmul(ps[:], lhsT,
                                         xh_t[:, 2 * j:2 * j + 2, :],
                                         start=(j == 0), stop=False,
                                         perf_mode=DR)
                        nc.tensor.matmul(ps[:], lhsT,
                                         xl_t[:, 2 * j:2 * j + 2, :],
                                         start=False, stop=(j == KP - 1),
                                         perf_mode=DR)
                else:
                    for j in range(KT):  # k-tile j
                        nc.tensor.matmul(ps[:], wq_sb[:, j, osl],
                                         xh_t[:, j, :],
                                         start=(j == 0), stop=(j == KT - 1))

            GROUPS = [(0, 1), (2, 3), (4, 5), (6, 7)]
            for grp in GROUPS:
                xts = [x_tiles(tb) for tb in grp]
                o_ts = [outp.tile([128, O_SUB, TB], dt.bfloat16,
                                  name=f"ot{i}", tag="ot")
                        for i in range(len(grp))]
                for osb in range(O_SUB):
                    for i in range(len(grp)):
                        ps = psp.tile([128, TB], dt.float32, name="ps",
                                      tag="ps")
                        mm_group(ps, xts[i], osb)
                        # out = b_small * psum + bias (per-partition bias)
                        nc.vector.tensor_scalar(o_ts[i][:, osb, :], ps[:],
                                                float(b_small),
                                                bias_sb[:, osb:osb + 1],
                                                Alu.mult, Alu.add)
                for i, tb in enumerate(grp):
                    nc.gpsimd.dma_start(oT_r[tb], o_ts[i][:])

    nc.compile()
    return nc


def kernel(x, weight, bias, basis):
    from concourse import bass_utils

    x = np.asarray(x, dtype=np.float32)
    weight = np.asarray(weight, dtype=np.float32)
    bias = np.asarray(bias, dtype=np.float32)
    basis = np.asarray(basis, dtype=np.float32)

    b_small, b_big = sorted(float(v) for v in np.abs(basis))
    mode = os.environ.get("LQ_MODE", "fp8dr")  # fp8dr | bf16
    fp8 = mode == "fp8dr"
    f8 = ml_dtypes.float8_e4m3
    bf16 = ml_dtypes.bfloat16

    # ---- host-side shard/layout prep (transpose, cast, slice)
    if fp8:
        xhf = x.astype(f8)
        xlf = (x - xhf.astype(np.float32)).astype(f8)
    else:
        xhf = x.astype(bf16)
        xlf = None

    KH = KT // 2

    def pack_x(arr, ts):
        # [T_SHARD, IN_F] -> [tb, p, kt, t]
        s = arr[ts * T_SHARD:(ts + 1) * T_SHARD]
        s = s.reshape(N_TB, TB, KT, 128)             # [tb, t, kt, p]
        return np.ascontiguousarray(s.transpose(0, 3, 2, 1))

    in_maps = []
    for c in range(N_CORES):
        c1, ts = divmod(c, N_TSH)
        wt = weight[c1 * O_SHARD:(c1 + 1) * O_SHARD, :].T  # [IN_F, O_SHARD]
        # -> [osb, half, p, kh*128]: per-osb column blocks, split in half-K
        wpk = np.ascontiguousarray(
            wt.reshape(2, KH, 128, O_SUB, 128).transpose(3, 0, 2, 1, 4)
            .reshape(O_SUB, 2, 128, KH * 128))
        m = {
            "wp": wpk,
            "xh": pack_x(xhf, ts),
            "bs": np.ascontiguousarray(
                bias[c1 * O_SHARD:(c1 + 1) * O_SHARD].reshape(O_SUB, 128).T),
        }
        if fp8:
            m["xl"] = pack_x(xlf, ts)
        in_maps.append(m)

    nc = _build_nc(b_small, b_big, mode)
    trace = os.environ.get("LQ_TRACE", "") == "1"
    res = bass_utils.run_bass_kernel_spmd(
        nc, in_maps, core_ids=list(range(N_CORES)), trace=trace)

    LAST_RUN_INFO.clear()
    LAST_RUN_INFO["exec_time_ns"] = res.exec_time_ns
    LAST_RUN_INFO["profile_json"] = res.profile_json
    LAST_RUN_INFO["nc"] = nc
    LAST_RUN_INFO["in_maps"] = in_maps

    out = np.empty((TOKENS, OUT_F), dtype=np.float32)
    for c in range(N_CORES):
        c1, ts = divmod(c, N_TSH)
        o = res.results[c]["oT"]  # [tb, p, osb*t] bf16
        o = np.asarray(o).astype(np.float32).reshape(N_TB, 128, O_SUB, TB)
        out[ts * T_SHARD:(ts + 1) * T_SHARD,
            c1 * O_SHARD:(c1 + 1) * O_SHARD] = (
            o.transpose(0, 3, 2, 1).reshape(T_SHARD, O_SHARD))
    return out


# revision 48
# speedup vs baseline: 1.1528x; 1.1528x over previous
"""LQLinear (2-bit learned VQ linear) Trainium2 kernel.

Math (Q_T=1): the least-squares basis refit only feeds the *discarded*
buffer update, so the forward output is

    out = x @ wq.T + bias

where wq bucketizes weight into the 4 sorted levels {+-b_small, +-3*b_small}
(b_big = 2*b_small for the reference basis), thresholds {-b_big, 0, +b_big}.

Device strategy (8 cores = 4 out-feature shards x 2 token shards):
  - per core: 1024 out rows x 4096 tokens, full K=4096.
  - wq = b_small * wqn, wqn = sign(w)*(R + sign(|w|-b_big)) in {+-1,+-3}
    (R = b_big/b_small = 2): EXACT in fp8e4m3. Quantize runs on ACT+DVE
    from f32 weights (f32 compare needed: bf16 weights misclassify
    ~3e-4 of weights at the +-b_big thresholds -> ~1.5% output error).
    The |w| vs b_big compare is done as w^2 vs b_big^2 so the abs step
    is a plain DVE multiply (walrus rejects abs_max in tensor ops);
    the final (ss2+R)*s_big is one fused DVE scalar_tensor_tensor.
    Quantize streams per-osb (128 output cols at a time, osb0 in
    quarter-K chunks) so the GEMM starts consuming wq after ~10us.
  - GEMM in fp8 DoubleRow perf mode (2 k-tiles per instruction, 0.5
    cyc/row = 157 TF/s): x split host-side into fp8 hi + fp8 lo
    (x = hi + lo to ~7 mantissa bits); both streams accumulate into one
    PSUM group. Measured rel err ~1.8e-3 incl. bf16 out (gate 2e-2).
    LQ_MODE=bf16 is a fallback single-stream bf16 path.
  - Token blocks run in pairs, osb-major inside the pair, so the PE has
    two token blocks of work per wq column block while phase A streams.
  - All DMAs are large contiguous per-partition chunks (8-16KB
    descriptors, 128 descriptors/transfer) on the Pool SWDGE ring
    (gpsimd) instead of the v1 pattern of ~78k 2KB descriptors on the
    SP HWDGE queue. SWDGE is the path with measured near-peak HBM
    bandwidth (341 GB/s at 1MB transfers); the v1 baseline measured
    ~118x over the cost model on real hardware, consistent with a
    per-descriptor HWDGE penalty.
  - DVE evicts PSUM with fused out = b_small*psum + bias[o], stored
    bf16 (halves output traffic; host upcasts).
  - Host prep is layout/cast-only sharding work: transpose/cast/slice.

TimelineSim cost-model estimate: ~289us vs 516us for the previous
f32r baseline (PE-bound at 82%; PE busy ~237us at the fp8 DoubleRow
rate). Total descriptors ~5.4k vs ~78k.
"""

import os
import sys

for _p in ("/opt/trn_rl_repo", "/root/.axon_site/_ro/trn_rl_repo"):
    if os.path.isdir(_p) and _p not in sys.path:
        sys.path.insert(0, _p)

import numpy as np
import ml_dtypes

N_CORES = 8
TOKENS = 8192
IN_F = 4096
OUT_F = 4096

N_OSH = 4                            # out-feature shards
N_TSH = 2                            # token shards
O_SHARD = OUT_F // N_OSH             # 1024 out rows per core
T_SHARD = TOKENS // N_TSH            # 4096 tokens per core
KT = IN_F // 128                     # 32 k-tiles
KP = KT // 2                         # 16 k-tile pairs (DoubleRow)
TB = 512                             # token block (psum free dim)
N_TB = T_SHARD // TB                 # 8 token blocks per core
O_SUB = O_SHARD // 128               # 8 output subtiles per core

LAST_RUN_INFO = {}


def _build_nc(b_small: float, b_big: float, mode: str):
    import concourse.bass as bass
    import concourse.mybir as mybir
    import concourse.tile as tile
    from concourse import bacc

    dt = mybir.dt
    Alu = mybir.AluOpType
    R = b_big / b_small

    fp8 = mode == "fp8dr"
    xdt = dt.float8e4 if fp8 else dt.bfloat16

    nc = bacc.Bacc("TRN2", target_bir_lowering=False, debug=False,
                   dynamic_dma_scratch_size=8192)

    KH = KT // 2                     # k-tiles per half-K chunk
    wp = nc.dram_tensor("wp", [O_SUB, 2, 128, KH * 128], dt.float32,
                        kind="ExternalInput")
    xh = nc.dram_tensor("xh", [N_TB, 128, KT * TB], xdt,
                        kind="ExternalInput")
    if fp8:
        xl = nc.dram_tensor("xl", [N_TB, 128, KT * TB], xdt,
                            kind="ExternalInput")
    bs = nc.dram_tensor("bs", [128, O_SUB], dt.float32, kind="ExternalInput")
    oT = nc.dram_tensor("oT", [N_TB, 128, O_SUB * TB], dt.bfloat16,
                        kind="ExternalOutput")

    wp_r = wp.ap()
    xh_r = xh.ap()
    if fp8:
        xl_r = xl.ap()
    oT_r = oT.ap()

    DR = mybir.MatmulPerfMode.DoubleRow

    with tile.TileContext(nc) as tc:
        with (
            tc.tile_pool(name="const", bufs=1) as const,
            tc.tile_pool(name="wq", bufs=1) as wqp,
            tc.tile_pool(name="wload", bufs=2) as wload,
            tc.tile_pool(name="quant", bufs=2) as qp,
            tc.tile_pool(name="xhp", bufs=3) as xhp,
            tc.tile_pool(name="xlp", bufs=3) as xlp,
            tc.tile_pool(name="outp", bufs=2) as outp,
            tc.tile_pool(name="psum", bufs=8, space="PSUM") as psp,
        ):
            bias_sb = const.tile([128, O_SUB], dt.float32)
            nc.sync.dma_start(bias_sb[:], bs.ap())
            # threshold as -b_big^2: |w| > b_big is evaluated as w^2 > b_big^2
            # so the |w| step can run on DVE as a plain multiply
            nbb = const.tile([128, 1], dt.float32, tag="nbb")
            nc.vector.memset(nbb[:], -float(b_big) * float(b_big))

            # ---- Phase A: quantize weight shard -> wqn {+-1,+-R} fp8/bf16,
            # [k-partition, kt, o] layout for (DoubleRow) lhsT slices.
            # Ordered per-osb so each 128-wide output-column block of wq
            # completes early; the GEMM consumes blocks while later ones
            # are still quantizing.
            wq_sb = wqp.tile([128, KT, O_SHARD], xdt)

            def quant_chunk(osb, k0, nk, dma_engine):
                # quantize w[k-tiles k0:k0+nk, osb block] -> wq
                w_t = wload.tile([128, nk, 128], dt.float32, name="w_t",
                                 tag="wl")
                dma_engine.dma_start(
                    w_t[:], wp_r[osb, k0 // KH, :,
                                 (k0 % KH) * 128:(k0 % KH + nk) * 128])
                sb = qp.tile([128, nk, 128], dt.float32, name="sb", tag="sb")
                av = qp.tile([128, nk, 128], dt.float32, name="av", tag="av")
                # ACT: s_big = sign(w); DVE or Pool: w^2;
                # ACT: ss2 = sign(w^2 - b_big^2)  (== sign(|w| - b_big));
                # DVE fused: wqn = (ss2 + R) * s_big in {+-(R-1), +-(R+1)}
                nc.scalar.sign(sb[:], w_t[:])
                nc.vector.tensor_tensor(av[:], w_t[:], w_t[:], Alu.mult)
                nc.scalar.sign(av[:], av[:], bias=nbb[:])
                nc.vector.scalar_tensor_tensor(
                    wq_sb[:, k0:k0 + nk, osb * 128:(osb + 1) * 128],
                    av[:], R, sb[:], Alu.add, Alu.mult)

            for osb in range(O_SUB):
                if osb == 0:
                    # quarter-K chunks: lower latency to the first wq block
                    for q in range(4):
                        quant_chunk(0, q * (KH // 2), KH // 2, nc.gpsimd)
                else:
                    quant_chunk(osb, 0, KH, nc.gpsimd)
                    quant_chunk(osb, KH, KH, nc.gpsimd)

            # ---- Phase B: GEMM psum[o128, t512] += wqn.T @ x, DoubleRow.
            # Token blocks run in pairs, osb-major inside the pair: the PE
            # has two token blocks of work per wq column block, so it keeps
            # up while quantization streams in behind it.
            def x_tiles(tb):
                t = xhp.tile([128, KT, TB], xdt, tag="xh")
                nc.gpsimd.dma_start(t[:], xh_r[tb])
                l = None
                if fp8:
                    l = xlp.tile([128, KT, TB], xdt, tag="xl")
                    nc.gpsimd.dma_start(l[:], xl_r[tb])
                return (t, l)

            def mm_group(ps, xt, osb):
                osl = slice(osb * 128, (osb + 1) * 128)
                xh_t, xl_t = xt
                if fp8:
                    for j in range(KP):  # ktpair j
                        lhsT = wq_sb[:, 2 * j:2 * j + 2, osl]
                        nc.tensor.matmul(ps[:], lhsT,
                                         xh_t[:, 2 * j:2 * j + 2, :],
                                         start=(j == 0), stop=False,
                                         perf_mode=DR)
                        nc.tensor.matmul(ps[:], lhsT,
                                         xl_t[:, 2 * j:2 * j + 2, :],
                                         start=False, stop=(j == KP - 1),
                                         perf_mode=DR)
                else:
                    for j in range(KT):  # k-tile j
                        nc.tensor.matmul(ps[:], wq_sb[:, j, osl],
                                         xh_t[:, j, :],
                                         start=(j == 0), stop=(j == KT - 1))

            GROUPS = [(0, 1), (2, 3), (4, 5), (6, 7)]
            for grp in GROUPS:
                xts = [x_tiles(tb) for tb in grp]
                o_ts = [outp.tile([128, O_SUB, TB], dt.bfloat16,
                                  name=f"ot{i}", tag="ot")
                        for i in range(len(grp))]
                for osb in range(O_SUB):
                    for i in range(len(grp)):
                        ps = psp.tile([128, TB], dt.float32, name="ps",
                                      tag="ps")
                        mm_group(ps, xts[i], osb)
                        # out = b_small * psum + bias (per-partition bias)
                        nc.vector.tensor_scalar(o_ts[i][:, osb, :], ps[:],
                                                float(b_small),
                                                bias_sb[:, osb:osb + 1],
                                                Alu.mult, Alu.add)
                for i, tb in enumerate(grp):
                    nc.gpsimd.dma_start(oT_r[tb], o_ts[i][:])

    nc.compile()
    return nc


def kernel(x, weight, bias, basis):
    from concourse import bass_utils

    x = np.asarray(x, dtype=np.float32)
    weight = np.asarray(weight, dtype=np.float32)
    bias = np.asarray(bias, dtype=np.float32)
    basis = np.asarray(basis, dtype=np.float32)

    b_small, b_big = sorted(float(v) for v in np.abs(basis))
    mode = os.environ.get("LQ_MODE", "fp8dr")  # fp8dr | bf16
    fp8 = mode == "fp8dr"
    f8 = ml_dtypes.float8_e4m3
    bf16 = ml_dtypes.bfloat16

    # ---- host-side shard/layout prep (transpose, cast, slice)
    if fp8:
        xhf = x.astype(f8)
        xlf = (x - xhf.astype(np.float32)).astype(f8)
    else:
        xhf = x.astype(bf16)
        xlf = None

    KH = KT // 2

    def pack_x(arr, ts):
        # [T_SHARD, IN_F] -> [tb, p, kt, t]
        s = arr[ts * T_SHARD:(ts + 1) * T_SHARD]
        s = s.reshape(N_TB, TB, KT, 128)             # [tb, t, kt, p]
        return np.ascontiguousarray(s.transpose(0, 3, 2, 1))

    in_maps = []
    for c in range(N_CORES):
        c1, ts = divmod(c, N_TSH)
        wt = weight[c1 * O_SHARD:(c1 + 1) * O_SHARD, :].T  # [IN_F, O_SHARD]
        # -> [osb, half, p, kh*128]: per-osb column blocks, split in half-K
        wpk = np.ascontiguousarray(
            wt.reshape(2, KH, 128, O_SUB, 128).transpose(3, 0, 2, 1, 4)
            .reshape(O_SUB, 2, 128, KH * 128))
        m = {
            "wp": wpk,
            "xh": pack_x(xhf, ts),
            "bs": np.ascontiguousarray(
                bias[c1 * O_SHARD:(c1 + 1) * O_SHARD].reshape(O_SUB, 128).T),
        }
        if fp8:
            m["xl"] = pack_x(xlf, ts)
        in_maps.append(m)

    nc = _build_nc(b_small, b_big, mode)
    trace = os.environ.get("LQ_TRACE", "") == "1"
    res = bass_utils.run_bass_kernel_spmd(
        nc, in_maps, core_ids=list(range(N_CORES)), trace=trace)

    LAST_RUN_INFO.clear()
    LAST_RUN_INFO["exec_time_ns"] = res.exec_time_ns
    LAST_RUN_INFO["profile_json"] = res.profile_json
    LAST_RUN_INFO["nc"] = nc
    LAST_RUN_INFO["in_maps"] = in_maps

    out = np.empty((TOKENS, OUT_F), dtype=np.float32)
    for c in range(N_CORES):
        c1, ts = divmod(c, N_TSH)
        o = res.results[c]["oT"]  # [tb, p, osb*t] bf16
        o = np.asarray(o).astype(np.float32).reshape(N_TB, 128, O_SUB, TB)
        out[ts * T_SHARD:(ts + 1) * T_SHARD,
            c1 * O_SHARD:(c1 + 1) * O_SHARD] = (
            o.transpose(0, 3, 2, 1).reshape(T_SHARD, O_SHARD))
    return out


# revision 55
# speedup vs baseline: 1.1580x; 1.0046x over previous
"""LQLinear (2-bit learned VQ linear) Trainium2 kernel.

Math (Q_T=1): the least-squares basis refit only feeds the *discarded*
buffer update, so the forward output is

    out = x @ wq.T + bias

where wq bucketizes weight into the 4 sorted levels {+-b_small, +-3*b_small}
(b_big = 2*b_small for the reference basis), thresholds {-b_big, 0, +b_big}.

Device strategy (8 cores = 4 out-feature shards x 2 token shards):
  - per core: 1024 out rows x 4096 tokens, full K=4096.
  - wq = b_small * wqn, wqn = sign(w)*(R + sign(|w|-b_big)) in {+-1,+-3}
    (R = b_big/b_small = 2): EXACT in fp8e4m3. Quantize runs on ACT+DVE
    from f32 weights (f32 compare needed: bf16 weights misclassify
    ~3e-4 of weights at the +-b_big thresholds -> ~1.5% output error).
    The |w| vs b_big compare is done as w^2 vs b_big^2 so the abs step
    is a plain DVE multiply (walrus rejects abs_max in tensor ops);
    the final (ss2+R)*s_big is one fused DVE scalar_tensor_tensor.
    Quantize streams per-osb (128 output cols at a time, osb0 in
    quarter-K chunks) so the GEMM starts consuming wq after ~10us.
  - GEMM in fp8 DoubleRow perf mode (2 k-tiles per instruction, 0.5
    cyc/row = 157 TF/s): x split host-side into fp8 hi + fp8 lo
    (x = hi + lo to ~7 mantissa bits); both streams accumulate into one
    PSUM group. Measured rel err ~1.8e-3 incl. bf16 out (gate 2e-2).
    LQ_MODE=bf16 is a fallback single-stream bf16 path.
  - Token blocks run in pairs, osb-major inside the pair, so the PE has
    two token blocks of work per wq column block while phase A streams.
  - All DMAs are large contiguous per-partition chunks (8-16KB
    descriptors, 128 descriptors/transfer) on the Pool SWDGE ring
    (gpsimd) instead of the v1 pattern of ~78k 2KB descriptors on the
    SP HWDGE queue. SWDGE is the path with measured near-peak HBM
    bandwidth (341 GB/s at 1MB transfers); the v1 baseline measured
    ~118x over the cost model on real hardware, consistent with a
    per-descriptor HWDGE penalty.
  - DVE evicts PSUM with fused out = b_small*psum + bias[o], stored
    bf16 (halves output traffic; host upcasts).
  - Host prep is layout/cast-only sharding work: transpose/cast/slice.

TimelineSim cost-model estimate: ~289us vs 516us for the previous
f32r baseline (PE-bound at 82%; PE busy ~237us at the fp8 DoubleRow
rate). Total descriptors ~5.4k vs ~78k.
"""

import os
import sys

for _p in ("/opt/trn_rl_repo", "/root/.axon_site/_ro/trn_rl_repo"):
    if os.path.isdir(_p) and _p not in sys.path:
        sys.path.insert(0, _p)

import numpy as np
import ml_dtypes

N_CORES = 8
TOKENS = 8192
IN_F = 4096
OUT_F = 4096

N_OSH = 4                            # out-feature shards
N_TSH = 2                            # token shards
O_SHARD = OUT_F // N_OSH             # 1024 out rows per core
T_SHARD = TOKENS // N_TSH            # 4096 tokens per core
KT = IN_F // 128                     # 32 k-tiles
KP = KT // 2                         # 16 k-tile pairs (DoubleRow)
TB = 512                             # token block (psum free dim)
N_TB = T_SHARD // TB                 # 8 token blocks per core
O_SUB = O_SHARD // 128               # 8 output subtiles per core

LAST_RUN_INFO = {}


def _build_nc(b_small: float, b_big: float, mode: str):
    import concourse.bass as bass
    import concourse.mybir as mybir
    import concourse.tile as tile
    from concourse import bacc

    dt = mybir.dt
    Alu = mybir.AluOpType
    R = b_big / b_small

    fp8 = mode == "fp8dr"
    xdt = dt.float8e4 if fp8 else dt.bfloat16

    nc = bacc.Bacc("TRN2", target_bir_lowering=False, debug=False,
                   dynamic_dma_scratch_size=8192)

    KH = KT // 2                     # k-tiles per half-K chunk
    wp = nc.dram_tensor("wp", [O_SUB, 2, 128, KH * 128], dt.float32,
                        kind="ExternalInput")
    xh = nc.dram_tensor("xh", [N_TB, 128, KT * TB], xdt,
                        kind="ExternalInput")
    if fp8:
        xl = nc.dram_tensor("xl", [N_TB, 128, KT * TB], xdt,
                            kind="ExternalInput")
    bs = nc.dram_tensor("bs", [128, O_SUB], dt.float32, kind="ExternalInput")
    oT = nc.dram_tensor("oT", [N_TB, 128, O_SUB * TB], dt.bfloat16,
                        kind="ExternalOutput")

    wp_r = wp.ap()
    xh_r = xh.ap()
    if fp8:
        xl_r = xl.ap()
    oT_r = oT.ap()

    DR = mybir.MatmulPerfMode.DoubleRow

    with tile.TileContext(nc) as tc:
        with (
            tc.tile_pool(name="const", bufs=1) as const,
            tc.tile_pool(name="wq", bufs=1) as wqp,
            tc.tile_pool(name="wload", bufs=2) as wload,
            tc.tile_pool(name="quant", bufs=2) as qp,
            tc.tile_pool(name="xhp", bufs=4) as xhp,
            tc.tile_pool(name="xlp", bufs=3) as xlp,
            tc.tile_pool(name="outp", bufs=2) as outp,
            tc.tile_pool(name="psum", bufs=8, space="PSUM") as psp,
        ):
            bias_sb = const.tile([128, O_SUB], dt.float32)
            nc.sync.dma_start(bias_sb[:], bs.ap())
            # threshold as -b_big^2: |w| > b_big is evaluated as w^2 > b_big^2
            # so the |w| step can run on DVE as a plain multiply
            nbb = const.tile([128, 1], dt.float32, tag="nbb")
            nc.vector.memset(nbb[:], -float(b_big) * float(b_big))

            # ---- Phase A: quantize weight shard -> wqn {+-1,+-R} fp8/bf16,
            # [k-partition, kt, o] layout for (DoubleRow) lhsT slices.
            # Ordered per-osb so each 128-wide output-column block of wq
            # completes early; the GEMM consumes blocks while later ones
            # are still quantizing.
            wq_sb = wqp.tile([128, KT, O_SHARD], xdt)

            def quant_chunk(osb, k0, nk, sq_eng):
                # quantize w[k-tiles k0:k0+nk, osb block] -> wq
                w_t = wload.tile([128, nk, 128], dt.float32, name="w_t",
                                 tag="wl")
                nc.gpsimd.dma_start(
                    w_t[:], wp_r[osb, k0 // KH, :,
                                 (k0 % KH) * 128:(k0 % KH + nk) * 128])
                sb = qp.tile([128, nk, 128], dt.float32, name="sb", tag="sb")
                av = qp.tile([128, nk, 128], dt.float32, name="av", tag="av")
                # ACT: s_big = sign(w); DVE (or Pool for h1): w^2;
                # ACT: ss2 = sign(w^2 - b_big^2)  (== sign(|w| - b_big));
                # DVE fused: wqn = (ss2 + R) * s_big in {+-(R-1), +-(R+1)}
                nc.scalar.sign(sb[:], w_t[:])
                sq_eng.tensor_tensor(av[:], w_t[:], w_t[:], Alu.mult)
                nc.scalar.sign(av[:], av[:], bias=nbb[:])
                nc.vector.scalar_tensor_tensor(
                    wq_sb[:, k0:k0 + nk, osb * 128:(osb + 1) * 128],
                    av[:], R, sb[:], Alu.add, Alu.mult)

            for osb in range(O_SUB):
                if osb == 0:
                    # quarter-K chunks: lower latency to the first wq block
                    for q in range(4):
                        quant_chunk(0, q * (KH // 2), KH // 2, nc.vector)
                else:
                    quant_chunk(osb, 0, KH, nc.vector)
                    quant_chunk(osb, KH, KH, nc.vector)

            # ---- Phase B: GEMM psum[o128, t512] += wqn.T @ x, DoubleRow.
            # Token blocks run in pairs, osb-major inside the pair: the PE
            # has two token blocks of work per wq column block, so it keeps
            # up while quantization streams in behind it.
            def x_tiles(tb):
                t = xhp.tile([128, KT, TB], xdt, tag="xh")
                nc.gpsimd.dma_start(t[:], xh_r[tb])
                l = None
                if fp8:
                    l = xlp.tile([128, KT, TB], xdt, tag="xl")
                    nc.gpsimd.dma_start(l[:], xl_r[tb])
                return (t, l)

            def mm_group(ps, xt, osb):
                osl = slice(osb * 128, (osb + 1) * 128)
                xh_t, xl_t = xt
                if fp8:
                    for j in range(KP):  # ktpair j
                        lhsT = wq_sb[:, 2 * j:2 * j + 2, osl]
                        nc.tensor.matmul(ps[:], lhsT,
                                         xh_t[:, 2 * j:2 * j + 2, :],
                                         start=(j == 0), stop=False,
                                         perf_mode=DR)
                        nc.tensor.matmul(ps[:], lhsT,
                                         xl_t[:, 2 * j:2 * j + 2, :],
                                         start=False, stop=(j == KP - 1),
                                         perf_mode=DR)
                else:
                    for j in range(KT):  # k-tile j
                        nc.tensor.matmul(ps[:], wq_sb[:, j, osl],
                                         xh_t[:, j, :],
                                         start=(j == 0), stop=(j == KT - 1))

            GROUPS = [(0, 1), (2, 3), (4, 5), (6, 7)]
            for grp in GROUPS:
                xts = [x_tiles(tb) for tb in grp]
                o_ts = [outp.tile([128, O_SUB, TB], dt.bfloat16,
                                  name=f"ot{i}", tag="ot")
                        for i in range(len(grp))]
                for osb in range(O_SUB):
                    for i in range(len(grp)):
                        ps = psp.tile([128, TB], dt.float32, name="ps",
                                      tag="ps")
                        mm_group(ps, xts[i], osb)
                        # out = b_small * psum + bias (per-partition bias)
                        nc.vector.tensor_scalar(o_ts[i][:, osb, :], ps[:],
                                                float(b_small),
                                                bias_sb[:, osb:osb + 1],
                                                Alu.mult, Alu.add)
                for i, tb in enumerate(grp):
                    nc.gpsimd.dma_start(oT_r[tb], o_ts[i][:])

    nc.compile()
    return nc


def kernel(x, weight, bias, basis):
    from concourse import bass_utils

    x = np.asarray(x, dtype=np.float32)
    weight = np.asarray(weight, dtype=np.float32)
    bias = np.asarray(bias, dtype=np.float32)
    basis = np.asarray(basis, dtype=np.float32)

    b_small, b_big = sorted(float(v) for v in np.abs(basis))
    mode = os.environ.get("LQ_MODE", "fp8dr")  # fp8dr | bf16
    fp8 = mode == "fp8dr"
    f8 = ml_dtypes.float8_e4m3
    bf16 = ml_dtypes.bfloat16

    # ---- host-side shard/layout prep (transpose, cast, slice)
    if fp8:
        xhf = x.astype(f8)
        xlf = (x - xhf.astype(np.float32)).astype(f8)
    else:
        xhf = x.astype(bf16)
        xlf = None

    KH = KT // 2

    def pack_x(arr, ts):
        # [T_SHARD, IN_F] -> [tb, p, kt, t]
        s = arr[ts * T_SHARD:(ts + 1) * T_SHARD]
        s = s.reshape(N_TB, TB, KT, 128)             # [tb, t, kt, p]
        return np.ascontiguousarray(s.transpose(0, 3, 2, 1))

    in_maps = []
    for c in range(N_CORES):
        c1, ts = divmod(c, N_TSH)
        wt = weight[c1 * O_SHARD:(c1 + 1) * O_SHARD, :].T  # [IN_F, O_SHARD]
        # -> [osb, half, p, kh*128]: per-osb column blocks, split in half-K
        wpk = np.ascontiguousarray(
            wt.reshape(2, KH, 128, O_SUB, 128).transpose(3, 0, 2, 1, 4)
            .reshape(O_SUB, 2, 128, KH * 128))
        m = {
            "wp": wpk,
            "xh": pack_x(xhf, ts),
            "bs": np.ascontiguousarray(
                bias[c1 * O_SHARD:(c1 + 1) * O_SHARD].reshape(O_SUB, 128).T),
        }
        if fp8:
            m["xl"] = pack_x(xlf, ts)
        in_maps.append(m)

    nc = _build_nc(b_small, b_big, mode)
    trace = os.environ.get("LQ_TRACE", "") == "1"
    res = bass_utils.run_bass_kernel_spmd(
        nc, in_maps, core_ids=list(range(N_CORES)), trace=trace)

    LAST_RUN_INFO.clear()
    LAST_RUN_INFO["exec_time_ns"] = res.exec_time_ns
    LAST_RUN_INFO["profile_json"] = res.profile_json
    LAST_RUN_INFO["nc"] = nc
    LAST_RUN_INFO["in_maps"] = in_maps

    out = np.empty((TOKENS, OUT_F), dtype=np.float32)
    for c in range(N_CORES):
        c1, ts = divmod(c, N_TSH)
        o = res.results[c]["oT"]  # [tb, p, osb*t] bf16
        o = np.asarray(o).astype(np.float32).reshape(N_TB, 128, O_SUB, TB)
        out[ts * T_SHARD:(ts + 1) * T_SHARD,
            c1 * O_SHARD:(c1 + 1) * O_SHARD] = (
            o.transpose(0, 3, 2, 1).reshape(T_SHARD, O_SHARD))
    return out


# revision 63
# speedup vs baseline: 1.1635x; 1.0047x over previous
"""LQLinear (2-bit learned VQ linear) Trainium2 kernel.

Math (Q_T=1): the least-squares basis refit only feeds the *discarded*
buffer update, so the forward output is

    out = x @ wq.T + bias

where wq bucketizes weight into the 4 sorted levels {+-b_small, +-3*b_small}
(b_big = 2*b_small for the reference basis), thresholds {-b_big, 0, +b_big}.

Device strategy (8 cores = 4 out-feature shards x 2 token shards):
  - per core: 1024 out rows x 4096 tokens, full K=4096.
  - wq = b_small * wqn, wqn = sign(w)*(R + sign(|w|-b_big)) in {+-1,+-3}
    (R = b_big/b_small = 2): EXACT in fp8e4m3. Quantize runs on ACT+DVE
    from f32 weights (f32 compare needed: bf16 weights misclassify
    ~3e-4 of weights at the +-b_big thresholds -> ~1.5% output error).
    The |w| vs b_big compare is done as w^2 vs b_big^2 so the abs step
    is a plain DVE multiply (walrus rejects abs_max in tensor ops);
    the final (ss2+R)*s_big is one fused DVE scalar_tensor_tensor.
    Quantize streams per-osb (128 output cols at a time, osb0 in
    quarter-K chunks) so the GEMM starts consuming wq after ~10us.
  - GEMM in fp8 DoubleRow perf mode (2 k-tiles per instruction, 0.5
    cyc/row = 157 TF/s): x split host-side into fp8 hi + fp8 lo
    (x = hi + lo to ~7 mantissa bits); both streams accumulate into one
    PSUM group. Measured rel err ~1.8e-3 incl. bf16 out (gate 2e-2).
    LQ_MODE=bf16 is a fallback single-stream bf16 path.
  - Token blocks run in pairs, osb-major inside the pair, so the PE has
    two token blocks of work per wq column block while phase A streams.
  - All DMAs are large contiguous per-partition chunks (8-16KB
    descriptors, 128 descriptors/transfer) on the Pool SWDGE ring
    (gpsimd) instead of the v1 pattern of ~78k 2KB descriptors on the
    SP HWDGE queue. SWDGE is the path with measured near-peak HBM
    bandwidth (341 GB/s at 1MB transfers); the v1 baseline measured
    ~118x over the cost model on real hardware, consistent with a
    per-descriptor HWDGE penalty.
  - DVE evicts PSUM with fused out = b_small*psum + bias[o], stored
    bf16 (halves output traffic; host upcasts).
  - Host prep is layout/cast-only sharding work: transpose/cast/slice.

TimelineSim cost-model estimate: ~288us vs 516us for the previous
f32r baseline (PE-bound at 82%; PE busy ~236us at the fp8 DoubleRow
rate). Total descriptors ~5.4k vs ~78k.
"""

import os
import sys

for _p in ("/opt/trn_rl_repo", "/root/.axon_site/_ro/trn_rl_repo"):
    if os.path.isdir(_p) and _p not in sys.path:
        sys.path.insert(0, _p)

import numpy as np
import ml_dtypes

N_CORES = 8
TOKENS = 8192
IN_F = 4096
OUT_F = 4096

N_OSH = 4                            # out-feature shards
N_TSH = 2                            # token shards
O_SHARD = OUT_F // N_OSH             # 1024 out rows per core
T_SHARD = TOKENS // N_TSH            # 4096 tokens per core
KT = IN_F // 128                     # 32 k-tiles
KP = KT // 2                         # 16 k-tile pairs (DoubleRow)
TB = 512                             # token block (psum free dim)
N_TB = T_SHARD // TB                 # 8 token blocks per core
O_SUB = O_SHARD // 128               # 8 output subtiles per core

LAST_RUN_INFO = {}


def _build_nc(b_small: float, b_big: float, mode: str):
    import concourse.bass as bass
    import concourse.mybir as mybir
    import concourse.tile as tile
    from concourse import bacc

    dt = mybir.dt
    Alu = mybir.AluOpType
    R = b_big / b_small

    fp8 = mode == "fp8dr"
    xdt = dt.float8e4 if fp8 else dt.bfloat16

    nc = bacc.Bacc("TRN2", target_bir_lowering=False, debug=False,
                   dynamic_dma_scratch_size=8192)

    KH = KT // 2                     # k-tiles per half-K chunk
    wp = nc.dram_tensor("wp", [O_SUB, 2, 128, KH * 128], dt.float32,
                        kind="ExternalInput")
    xh = nc.dram_tensor("xh", [N_TB, 128, KT * TB], xdt,
                        kind="ExternalInput")
    if fp8:
        xl = nc.dram_tensor("xl", [N_TB, 128, KT * TB], xdt,
                            kind="ExternalInput")
    bs = nc.dram_tensor("bs", [128, O_SUB], dt.float32, kind="ExternalInput")
    oT = nc.dram_tensor("oT", [N_TB, 128, O_SUB * TB], dt.bfloat16,
                        kind="ExternalOutput")

    wp_r = wp.ap()
    xh_r = xh.ap()
    if fp8:
        xl_r = xl.ap()
    oT_r = oT.ap()

    DR = mybir.MatmulPerfMode.DoubleRow

    with tile.TileContext(nc) as tc:
        with (
            tc.tile_pool(name="const", bufs=1) as const,
            tc.tile_pool(name="wq", bufs=1) as wqp,
            tc.tile_pool(name="wload", bufs=2) as wload,
            tc.tile_pool(name="quant", bufs=2) as qp,
            tc.tile_pool(name="xhp", bufs=4) as xhp,
            tc.tile_pool(name="xlp", bufs=3) as xlp,
            tc.tile_pool(name="outp", bufs=2) as outp,
            tc.tile_pool(name="psum", bufs=8, space="PSUM") as psp,
        ):
            bias_sb = const.tile([128, O_SUB], dt.float32)
            nc.sync.dma_start(bias_sb[:], bs.ap())
            # threshold as -b_big^2: |w| > b_big is evaluated as w^2 > b_big^2
            # so the |w| step can run on DVE as a plain multiply
            nbb = const.tile([128, 1], dt.float32, tag="nbb")
            nc.vector.memset(nbb[:], -float(b_big) * float(b_big))

            # ---- Phase A: quantize weight shard -> wqn {+-1,+-R} fp8/bf16,
            # [k-partition, kt, o] layout for (DoubleRow) lhsT slices.
            # Ordered per-osb so each 128-wide output-column block of wq
            # completes early; the GEMM consumes blocks while later ones
            # are still quantizing.
            wq_sb = wqp.tile([128, KT, O_SHARD], xdt)

            def quant_chunk(osb, k0, nk, sq_eng):
                # quantize w[k-tiles k0:k0+nk, osb block] -> wq
                w_t = wload.tile([128, nk, 128], dt.float32, name="w_t",
                                 tag="wl")
                nc.gpsimd.dma_start(
                    w_t[:], wp_r[osb, k0 // KH, :,
                                 (k0 % KH) * 128:(k0 % KH + nk) * 128])
                sb = qp.tile([128, nk, 128], dt.float32, name="sb", tag="sb")
                av = qp.tile([128, nk, 128], dt.float32, name="av", tag="av")
                # ACT: s_big = sign(w); DVE (or Pool for h1): w^2;
                # ACT: ss2 = sign(w^2 - b_big^2)  (== sign(|w| - b_big));
                # DVE fused: wqn = (ss2 + R) * s_big in {+-(R-1), +-(R+1)}
                nc.scalar.sign(sb[:], w_t[:])
                sq_eng.tensor_tensor(av[:], w_t[:], w_t[:], Alu.mult)
                nc.scalar.sign(av[:], av[:], bias=nbb[:])
                nc.vector.scalar_tensor_tensor(
                    wq_sb[:, k0:k0 + nk, osb * 128:(osb + 1) * 128],
                    av[:], R, sb[:], Alu.add, Alu.mult)

            for osb in range(O_SUB):
                if osb == 0:
                    # quarter-K chunks: lower latency to the first wq block
                    for q in range(4):
                        quant_chunk(0, q * (KH // 2), KH // 2, nc.vector)
                else:
                    quant_chunk(osb, 0, KH, nc.vector)
                    quant_chunk(osb, KH, KH, nc.vector)

            # ---- Phase B: GEMM psum[o128, t512] += wqn.T @ x, DoubleRow.
            # Token blocks run in pairs, osb-major inside the pair: the PE
            # has two token blocks of work per wq column block, so it keeps
            # up while quantization streams in behind it.
            def x_tiles(tb):
                t = xhp.tile([128, KT, TB], xdt, tag="xh")
                nc.gpsimd.dma_start(t[:], xh_r[tb])
                l = None
                if fp8:
                    l = xlp.tile([128, KT, TB], xdt, tag="xl")
                    nc.gpsimd.dma_start(l[:], xl_r[tb])
                return (t, l)

            def mm_group(ps, xt, osb):
                osl = slice(osb * 128, (osb + 1) * 128)
                xh_t, xl_t = xt
                if fp8:
                    for j in range(KP):  # ktpair j
                        lhsT = wq_sb[:, 2 * j:2 * j + 2, osl]
                        nc.tensor.matmul(ps[:], lhsT,
                                         xh_t[:, 2 * j:2 * j + 2, :],
                                         start=(j == 0), stop=False,
                                         perf_mode=DR)
                        nc.tensor.matmul(ps[:], lhsT,
                                         xl_t[:, 2 * j:2 * j + 2, :],
                                         start=False, stop=(j == KP - 1),
                                         perf_mode=DR)
                else:
                    for j in range(KT):  # k-tile j
                        nc.tensor.matmul(ps[:], wq_sb[:, j, osl],
                                         xh_t[:, j, :],
                                         start=(j == 0), stop=(j == KT - 1))

            GROUPS = [(0, 1), (2, 3), (4, 5), (6, 7)]
            for grp in GROUPS:
                xts = [x_tiles(tb) for tb in grp]
                o_ts = [outp.tile([128, O_SUB, TB], dt.bfloat16,
                                  name=f"ot{i}", tag="ot")
                        for i in range(len(grp))]
                for osb in range(O_SUB):
                    for i in range(len(grp)):
                        ps = psp.tile([128, TB], dt.float32, name="ps",
                                      tag="ps")
                        mm_group(ps, xts[i], osb)
                        # out = b_small * psum + bias (per-partition bias)
                        nc.vector.tensor_scalar(o_ts[i][:, osb, :], ps[:],
                                                float(b_small),
                                                bias_sb[:, osb:osb + 1],
                                                Alu.mult, Alu.add)
                if grp == GROUPS[-1]:
                    # tail: store the first half as soon as osb3 is evicted
                    for i, tb in enumerate(grp):
                        nc.gpsimd.dma_start(
                            oT_r[tb, :, :O_SUB * TB // 2],
                            o_ts[i][:, :O_SUB // 2, :])
                    for i, tb in enumerate(grp):
                        nc.gpsimd.dma_start(
                            oT_r[tb, :, O_SUB * TB // 2:],
                            o_ts[i][:, O_SUB // 2:, :])
                else:
                    for i, tb in enumerate(grp):
                        nc.gpsimd.dma_start(oT_r[tb], o_ts[i][:])

    nc.compile()
    return nc


def kernel(x, weight, bias, basis):
    from concourse import bass_utils

    x = np.asarray(x, dtype=np.float32)
    weight = np.asarray(weight, dtype=np.float32)
    bias = np.asarray(bias, dtype=np.float32)
    basis = np.asarray(basis, dtype=np.float32)

    b_small, b_big = sorted(float(v) for v in np.abs(basis))
    mode = os.environ.get("LQ_MODE", "fp8dr")  # fp8dr | bf16
    fp8 = mode == "fp8dr"
    f8 = ml_dtypes.float8_e4m3
    bf16 = ml_dtypes.bfloat16

    # ---- host-side shard/layout prep (transpose, cast, slice)
    if fp8:
        xhf = x.astype(f8)
        xlf = (x - xhf.astype(np.float32)).astype(f8)
    else:
        xhf = x.astype(bf16)
        xlf = None

    KH = KT // 2

    def pack_x(arr, ts):
        # [T_SHARD, IN_F] -> [tb, p, kt, t]
        s = arr[ts * T_SHARD:(ts + 1) * T_SHARD]
        s = s.reshape(N_TB, TB, KT, 128)             # [tb, t, kt, p]
        return np.ascontiguousarray(s.transpose(0, 3, 2, 1))

    in_maps = []
    for c in range(N_CORES):
        c1, ts = divmod(c, N_TSH)
        wt = weight[c1 * O_SHARD:(c1 + 1) * O_SHARD, :].T  # [IN_F, O_SHARD]
        # -> [osb, half, p, kh*128]: per-osb column blocks, split in half-K
        wpk = np.ascontiguousarray(
            wt.reshape(2, KH, 128, O_SUB, 128).transpose(3, 0, 2, 1, 4)
            .reshape(O_SUB, 2, 128, KH * 128))
        m = {
            "wp": wpk,
            "xh": pack_x(xhf, ts),
            "bs": np.ascontiguousarray(
                bias[c1 * O_SHARD:(c1 + 1) * O_SHARD].reshape(O_SUB, 128).T),
        }
        if fp8:
            m["xl"] = pack_x(xlf, ts)
        in_maps.append(m)

    nc = _build_nc(b_small, b_big, mode)
    trace = os.environ.get("LQ_TRACE", "") == "1"
    res = bass_utils.run_bass_kernel_spmd(
        nc, in_maps, core_ids=list(range(N_CORES)), trace=trace)

    LAST_RUN_INFO.clear()
    LAST_RUN_INFO["exec_time_ns"] = res.exec_time_ns
    LAST_RUN_INFO["profile_json"] = res.profile_json
    LAST_RUN_INFO["nc"] = nc
    LAST_RUN_INFO["in_maps"] = in_maps

    out = np.empty((TOKENS, OUT_F), dtype=np.float32)
    for c in range(N_CORES):
        c1, ts = divmod(c, N_TSH)
        o = res.results[c]["oT"]  # [tb, p, osb*t] bf16
        o = np.asarray(o).astype(np.float32).reshape(N_TB, 128, O_SUB, TB)
        out[ts * T_SHARD:(ts + 1) * T_SHARD,
            c1 * O_SHARD:(c1 + 1) * O_SHARD] = (
            o.transpose(0, 3, 2, 1).reshape(T_SHARD, O_SHARD))
    return out


# revision 65
# speedup vs baseline: 1.1854x; 1.0188x over previous
"""LQLinear (2-bit learned VQ linear) Trainium2 kernel.

Math (Q_T=1): the least-squares basis refit only feeds the *discarded*
buffer update, so the forward output is

    out = x @ wq.T + bias

where wq bucketizes weight into the 4 sorted levels {+-b_small, +-3*b_small}
(b_big = 2*b_small for the reference basis), thresholds {-b_big, 0, +b_big}.

Device strategy (8 cores = 4 out-feature shards x 2 token shards):
  - per core: 1024 out rows x 4096 tokens, full K=4096.
  - wq = b_small * wqn, wqn = sign(w)*(R + sign(|w|-b_big)) in {+-1,+-3}
    (R = b_big/b_small = 2): EXACT in fp8e4m3. Quantize runs on ACT+DVE
    from f32 weights (f32 compare needed: bf16 weights misclassify
    ~3e-4 of weights at the +-b_big thresholds -> ~1.5% output error).
    The |w| vs b_big compare is done as w^2 vs b_big^2 so the abs step
    is a plain DVE multiply (walrus rejects abs_max in tensor ops);
    the final (ss2+R)*s_big is one fused DVE scalar_tensor_tensor.
    Quantize streams per-osb (128 output cols at a time, osb0 in
    quarter-K chunks) so the GEMM starts consuming wq after ~10us.
  - GEMM in fp8 DoubleRow perf mode (2 k-tiles per instruction, 0.5
    cyc/row = 157 TF/s): x split host-side into fp8 hi + fp8 lo
    (x = hi + lo to ~7 mantissa bits); both streams accumulate into one
    PSUM group. Measured rel err ~1.8e-3 incl. bf16 out (gate 2e-2).
    LQ_MODE=bf16 is a fallback single-stream bf16 path.
  - Token blocks run in pairs, osb-major inside the pair, so the PE has
    two token blocks of work per wq column block while phase A streams.
  - All DMAs are large contiguous per-partition chunks (8-16KB
    descriptors, 128 descriptors/transfer) on the Pool SWDGE ring
    (gpsimd) instead of the v1 pattern of ~78k 2KB descriptors on the
    SP HWDGE queue. SWDGE is the path with measured near-peak HBM
    bandwidth (341 GB/s at 1MB transfers); the v1 baseline measured
    ~118x over the cost model on real hardware, consistent with a
    per-descriptor HWDGE penalty.
  - DVE evicts PSUM with fused out = b_small*psum + bias[o], stored
    bf16 (halves output traffic; host upcasts).
  - Host prep is layout/cast-only sharding work: transpose/cast/slice.

TimelineSim cost-model estimate: ~286us vs 516us for the previous
f32r baseline (PE-bound at 82%; PE busy ~236us at the fp8 DoubleRow
rate). Total descriptors ~5.4k vs ~78k.
"""

import os
import sys

for _p in ("/opt/trn_rl_repo", "/root/.axon_site/_ro/trn_rl_repo"):
    if os.path.isdir(_p) and _p not in sys.path:
        sys.path.insert(0, _p)

import numpy as np
import ml_dtypes

N_CORES = 8
TOKENS = 8192
IN_F = 4096
OUT_F = 4096

N_OSH = 4                            # out-feature shards
N_TSH = 2                            # token shards
O_SHARD = OUT_F // N_OSH             # 1024 out rows per core
T_SHARD = TOKENS // N_TSH            # 4096 tokens per core
KT = IN_F // 128                     # 32 k-tiles
KP = KT // 2                         # 16 k-tile pairs (DoubleRow)
TB = 512                             # token block (psum free dim)
N_TB = T_SHARD // TB                 # 8 token blocks per core
O_SUB = O_SHARD // 128               # 8 output subtiles per core

LAST_RUN_INFO = {}


def _build_nc(b_small: float, b_big: float, mode: str):
    import concourse.bass as bass
    import concourse.mybir as mybir
    import concourse.tile as tile
    from concourse import bacc

    dt = mybir.dt
    Alu = mybir.AluOpType
    R = b_big / b_small

    fp8 = mode == "fp8dr"
    xdt = dt.float8e4 if fp8 else dt.bfloat16

    nc = bacc.Bacc("TRN2", target_bir_lowering=False, debug=False,
                   dynamic_dma_scratch_size=8192)

    KH = KT // 2                     # k-tiles per half-K chunk
    wp = nc.dram_tensor("wp", [O_SUB, 2, 128, KH * 128], dt.float32,
                        kind="ExternalInput")
    xh = nc.dram_tensor("xh", [N_TB, 128, KT * TB], xdt,
                        kind="ExternalInput")
    if fp8:
        xl = nc.dram_tensor("xl", [N_TB, 128, KT * TB], xdt,
                            kind="ExternalInput")
    bs = nc.dram_tensor("bs", [128, O_SUB], dt.float32, kind="ExternalInput")
    oT = nc.dram_tensor("oT", [N_TB, 128, O_SUB * TB], dt.bfloat16,
                        kind="ExternalOutput")

    wp_r = wp.ap()
    xh_r = xh.ap()
    if fp8:
        xl_r = xl.ap()
    oT_r = oT.ap()

    DR = mybir.MatmulPerfMode.DoubleRow

    with tile.TileContext(nc) as tc:
        with (
            tc.tile_pool(name="const", bufs=1) as const,
            tc.tile_pool(name="wq", bufs=1) as wqp,
            tc.tile_pool(name="wload", bufs=2) as wload,
            tc.tile_pool(name="quant", bufs=2) as qp,
            tc.tile_pool(name="xhp", bufs=4) as xhp,
            tc.tile_pool(name="xlp", bufs=3) as xlp,
            tc.tile_pool(name="outp", bufs=2) as outp,
            tc.tile_pool(name="psum", bufs=8, space="PSUM") as psp,
        ):
            bias_sb = const.tile([128, O_SUB], dt.float32)
            nc.sync.dma_start(bias_sb[:], bs.ap())
            # threshold as -b_big^2: |w| > b_big is evaluated as w^2 > b_big^2
            # so the |w| step can run on DVE as a plain multiply
            nbb = const.tile([128, 1], dt.float32, tag="nbb")
            nc.vector.memset(nbb[:], -float(b_big) * float(b_big))

            # ---- Phase A: quantize weight shard -> wqn {+-1,+-R} fp8/bf16,
            # [k-partition, kt, o] layout for (DoubleRow) lhsT slices.
            # Ordered per-osb so each 128-wide output-column block of wq
            # completes early; the GEMM consumes blocks while later ones
            # are still quantizing.
            wq_sb = wqp.tile([128, KT, O_SHARD], xdt)

            def quant_chunk(osb, k0, nk, sq_eng):
                # quantize w[k-tiles k0:k0+nk, osb block] -> wq
                w_t = wload.tile([128, nk, 128], dt.float32, name="w_t",
                                 tag="wl")
                nc.gpsimd.dma_start(
                    w_t[:], wp_r[osb, k0 // KH, :,
                                 (k0 % KH) * 128:(k0 % KH + nk) * 128])
                sb = qp.tile([128, nk, 128], dt.float32, name="sb", tag="sb")
                av = qp.tile([128, nk, 128], dt.float32, name="av", tag="av")
                # ACT: s_big = sign(w); DVE (or Pool for h1): w^2;
                # ACT: ss2 = sign(w^2 - b_big^2)  (== sign(|w| - b_big));
                # DVE fused: wqn = (ss2 + R) * s_big in {+-(R-1), +-(R+1)}
                nc.scalar.sign(sb[:], w_t[:])
                sq_eng.tensor_tensor(av[:], w_t[:], w_t[:], Alu.mult)
                nc.scalar.sign(av[:], av[:], bias=nbb[:])
                nc.vector.scalar_tensor_tensor(
                    wq_sb[:, k0:k0 + nk, osb * 128:(osb + 1) * 128],
                    av[:], R, sb[:], Alu.add, Alu.mult)

            for osb in range(O_SUB):
                if osb <= 1:
                    # quarter-K chunks: lower latency to the first wq blocks
                    for q in range(4):
                        quant_chunk(osb, q * (KH // 2), KH // 2, nc.vector)
                else:
                    quant_chunk(osb, 0, KH, nc.vector)
                    quant_chunk(osb, KH, KH, nc.vector)

            # ---- Phase B: GEMM psum[o128, t512] += wqn.T @ x, DoubleRow.
            # Token blocks run in pairs, osb-major inside the pair: the PE
            # has two token blocks of work per wq column block, so it keeps
            # up while quantization streams in behind it.
            def x_tiles(tb):
                t = xhp.tile([128, KT, TB], xdt, tag="xh")
                nc.gpsimd.dma_start(t[:], xh_r[tb])
                l = None
                if fp8:
                    l = xlp.tile([128, KT, TB], xdt, tag="xl")
                    nc.gpsimd.dma_start(l[:], xl_r[tb])
                return (t, l)

            def mm_group(ps, xt, osb):
                osl = slice(osb * 128, (osb + 1) * 128)
                xh_t, xl_t = xt
                if fp8:
                    for j in range(KP):  # ktpair j
                        lhsT = wq_sb[:, 2 * j:2 * j + 2, osl]
                        nc.tensor.matmul(ps[:], lhsT,
                                         xh_t[:, 2 * j:2 * j + 2, :],
                                         start=(j == 0), stop=False,
                                         perf_mode=DR)
                        nc.tensor.matmul(ps[:], lhsT,
                                         xl_t[:, 2 * j:2 * j + 2, :],
                                         start=False, stop=(j == KP - 1),
                                         perf_mode=DR)
                else:
                    for j in range(KT):  # k-tile j
                        nc.tensor.matmul(ps[:], wq_sb[:, j, osl],
                                         xh_t[:, j, :],
                                         start=(j == 0), stop=(j == KT - 1))

            GROUPS = [(0, 1), (2, 3), (4, 5), (6, 7)]
            for grp in GROUPS:
                xts = [x_tiles(tb) for tb in grp]
                o_ts = [outp.tile([128, O_SUB, TB], dt.bfloat16,
                                  name=f"ot{i}", tag="ot")
                        for i in range(len(grp))]
                for osb in range(O_SUB):
                    for i in range(len(grp)):
                        ps = psp.tile([128, TB], dt.float32, name="ps",
                                      tag="ps")
                        mm_group(ps, xts[i], osb)
                        # out = b_small * psum + bias (per-partition bias)
                        nc.vector.tensor_scalar(o_ts[i][:, osb, :], ps[:],
                                                float(b_small),
                                                bias_sb[:, osb:osb + 1],
                                                Alu.mult, Alu.add)
                if grp == GROUPS[-1]:
                    # tail: store the first half as soon as osb3 is evicted
                    for i, tb in enumerate(grp):
                        nc.gpsimd.dma_start(
                            oT_r[tb, :, :O_SUB * TB // 2],
                            o_ts[i][:, :O_SUB // 2, :])
                    for i, tb in enumerate(grp):
                        nc.gpsimd.dma_start(
                            oT_r[tb, :, O_SUB * TB // 2:],
                            o_ts[i][:, O_SUB // 2:, :])
                else:
                    for i, tb in enumerate(grp):
                        nc.gpsimd.dma_start(oT_r[tb], o_ts[i][:])

    nc.compile()
    return nc


def kernel(x, weight, bias, basis):
    from concourse import bass_utils

    x = np.asarray(x, dtype=np.float32)
    weight = np.asarray(weight, dtype=np.float32)
    bias = np.asarray(bias, dtype=np.float32)
    basis = np.asarray(basis, dtype=np.float32)

    b_small, b_big = sorted(float(v) for v in np.abs(basis))
    mode = os.environ.get("LQ_MODE", "fp8dr")  # fp8dr | bf16
    fp8 = mode == "fp8dr"
    f8 = ml_dtypes.float8_e4m3
    bf16 = ml_dtypes.bfloat16

    # ---- host-side shard/layout prep (transpose, cast, slice)
    if fp8:
        xhf = x.astype(f8)
        xlf = (x - xhf.astype(np.float32)).astype(f8)
    else:
        xhf = x.astype(bf16)
        xlf = None

    KH = KT // 2

    def pack_x(arr, ts):
        # [T_SHARD, IN_F] -> [tb, p, kt, t]
        s = arr[ts * T_SHARD:(ts + 1) * T_SHARD]
        s = s.reshape(N_TB, TB, KT, 128)             # [tb, t, kt, p]
        return np.ascontiguousarray(s.transpose(0, 3, 2, 1))

    in_maps = []
    for c in range(N_CORES):
        c1, ts = divmod(c, N_TSH)
        wt = weight[c1 * O_SHARD:(c1 + 1) * O_SHARD, :].T  # [IN_F, O_SHARD]
        # -> [osb, half, p, kh*128]: per-osb column blocks, split in half-K
        wpk = np.ascontiguousarray(
            wt.reshape(2, KH, 128, O_SUB, 128).transpose(3, 0, 2, 1, 4)
            .reshape(O_SUB, 2, 128, KH * 128))
        m = {
            "wp": wpk,
            "xh": pack_x(xhf, ts),
            "bs": np.ascontiguousarray(
                bias[c1 * O_SHARD:(c1 + 1) * O_SHARD].reshape(O_SUB, 128).T),
        }
        if fp8:
            m["xl"] = pack_x(xlf, ts)
        in_maps.append(m)

    nc = _build_nc(b_small, b_big, mode)
    trace = os.environ.get("LQ_TRACE", "") == "1"
    res = bass_utils.run_bass_kernel_spmd(
        nc, in_maps, core_ids=list(range(N_CORES)), trace=trace)

    LAST_RUN_INFO.clear()
    LAST_RUN_INFO["exec_time_ns"] = res.exec_time_ns
    LAST_RUN_INFO["profile_json"] = res.profile_json
    LAST_RUN_INFO["nc"] = nc
    LAST_RUN_INFO["in_maps"] = in_maps

    out = np.empty((TOKENS, OUT_F), dtype=np.float32)
    for c in range(N_CORES):
        c1, ts = divmod(c, N_TSH)
        o = res.results[c]["oT"]  # [tb, p, osb*t] bf16
        o = np.asarray(o).astype(np.float32).reshape(N_TB, 128, O_SUB, TB)
        out[ts * T_SHARD:(ts + 1) * T_SHARD,
            c1 * O_SHARD:(c1 + 1) * O_SHARD] = (
            o.transpose(0, 3, 2, 1).reshape(T_SHARD, O_SHARD))
    return out
